# revision 20
# baseline (speedup 1.0000x reference)
"""AnchorGenerator Bass kernel for 8 Trainium2 NeuronCores.

Generates multi-level anchor boxes: for each of 4 feature-map levels
(stride 4/8/16/32, sizes 1024^2/512^2/256^2/128^2), the output is
[9*h*w, 4] f32 rows (cx, cy, aw, ah), ordered (anchor_type, y, x).

Strategy (memory-bound, pure output generation ~200MB):
  - Shard each level's flattened rows contiguously across the 8 cores:
    core k owns rows [k*N/8, (k+1)*N/8). Host gather is a plain
    concatenate.
  - Per core+level the shard is [G8 row-groups, 4w floats]. Large levels
    are emitted as [128, 4w] tiles (2MB DMAs for level 0); the two small
    levels are packed into a single SBUF buffer each ([PP, S*4w]: each
    partition holds S consecutive row-groups) so each goes out as one
    DMA.
  - Tile content: col0 = xc (x centers, from the table input), cols
    1/2/3 (y center / anchor w / anchor h) are per-row-group constants
    written via per-partition-scalar ops (vector engine carries 3 of
    the 4 column writes, scalar engine 1).
  - Levels are emitted smallest-first so the small (slow, per-byte)
    transfers overlap the pipeline ramp and the kernel tail is big
    fast DMAs.
"""

import numpy as np

import concourse.bass as bass
import concourse.bacc as bacc
import concourse.mybir as mybir
from concourse.tile import TileContext
from concourse.bass_utils import run_bass_kernel_spmd

NCORES = 8
STRIDES = [4, 8, 16, 32]
SIZES = [(1024, 1024), (512, 512), (256, 256), (128, 128)]
S = 3  # scales
R = 3  # ratios
A = S * R

# Per-level derived constants, in EMISSION order (small -> large):
#   (lvl, h, w, stride, G8, PP, SLOTS)
#   G8 = row-groups (of w rows) per core; the shard is laid out as
#   [PP partitions, SLOTS row-groups each]; PP*SLOTS == G8 for packed
#   levels; SLOTS==1 means 128-partition tiling with T=ceil(G8/128).
_PLAN = [  # lvl, PP, SLOTS
    (3, 72, 2),    # G8=144  -> one [72, 4KB] buffer, one DMA
    (2, 96, 3),    # G8=288  -> one [96, 12KB] buffer, one DMA
    (1, 128, 1),   # G8=576  -> 5 [128, 8KB] tiles
    (0, 128, 1),   # G8=1152 -> 9 [128, 16KB] tiles
]
LEVELS = []
for _lvl, _PP, _SL in _PLAN:
    _h, _w = SIZES[_lvl]
    _G8 = A * _h // NCORES
    assert _h % 128 == 0 and (_SL == 1 or _PP * _SL == _G8)
    LEVELS.append((_lvl, _h, _w, STRIDES[_lvl], _G8, _PP, _SL))

# Tiled (SLOTS==1) levels use 124-partition tiles: SDMA engine 15 — the
# one that is intermittently slow on TRN2 — serves SBUF partitions
# {92-95, 124-127}, so avoiding partitions 124-127 halves its share of
# every large transfer while the per-engine max stays the same.
TP = 124

# Table input layout: per level (emission order) a contiguous chunk:
#   [3 scalar cols per column-group ... | xc (w values)]
# where a column-group is one (tile) for SLOTS==1 levels (T of them) or
# one slot for packed levels (SLOTS of them).
CHUNKS = []  # (offset, ngroups, xc_off, width)
_o = 0
for (_lvl, _h, _w, _s, _G8, _PP, _SL) in LEVELS:
    _ng = _SL if _SL > 1 else (_G8 + TP - 1) // TP
    CHUNKS.append((_o, _ng, _o + 3 * _ng, _w))
    _o += 3 * _ng + _w
SCALW = _o

NBUFS = 9  # work-tile slots for the SLOTS==1 levels

_F32 = mybir.dt.float32

_BUILT = None  # cached build


def _build():
    """Build the Bass program (identical on all 8 cores; per-core data
    arrives via the small scalar-table input)."""
    nc = bacc.Bacc()

    scal = nc.dram_tensor("scal", [128, SCALW], _F32, kind="ExternalInput")
    outs = {}
    for (lvl, h, w, stride, G8, PP, SL) in LEVELS:
        outs[lvl] = nc.dram_tensor(
            f"out{lvl}", [G8, 4 * w], _F32, kind="ExternalOutput"
        )

    ident = mybir.ActivationFunctionType.Identity
    mult, add = mybir.AluOpType.mult, mybir.AluOpType.add

    def fill(tv, xcs, sc, c0, P):
        """Write one row-group-column: col0=xc, col1/2/3 = per-partition
        constants from table columns c0..c0+2."""
        nc.vector.tensor_copy(tv[:, :, 0], xcs)
        nc.vector.tensor_scalar(
            tv[:, :, 1], xcs, 0.0, sc[:P, c0 : c0 + 1], mult, add
        )
        nc.vector.tensor_scalar(
            tv[:, :, 2], xcs, 0.0, sc[:P, c0 + 1 : c0 + 2], mult, add
        )
        nc.scalar.activation(
            tv[:, :, 3], xcs, ident, bias=sc[:P, c0 + 2 : c0 + 3], scale=0.0
        )

    # Emission schedule: front-load big L0 tiles so the DMA queue
    # saturates with 2MB transfers as early as possible; interleave L1
    # and drop the packed L2/L3 buffers mid-stream.
    def _schedule():
        li = {lvl: i for i, (lvl, *_rest) in enumerate(LEVELS)}
        nt = {lvl: (G8 + TP - 1) // TP
              for (lvl, h, w, s, G8, PP, SL) in LEVELS}
        sched = [("p", li[3], 0), ("p", li[2], 0)]
        # partial (last) tiles first so their lower-parallelism transfers
        # land during the ramp, not the saturated middle/tail
        sched += [("t", li[1], t)
                  for t in [nt[1] - 1] + list(range(nt[1] - 1))]
        sched += [("t", li[0], t)
                  for t in [nt[0] - 1] + list(range(nt[0] - 1))]
        return sched

    with TileContext(nc) as tc:
        with (
            tc.tile_pool(name="consts", bufs=1) as cpool,
            tc.tile_pool(name="work", bufs=NBUFS) as wpool,
        ):
            sc = cpool.tile([128, SCALW], _F32, tag="sc", name="sc")
            # per-level table chunks on the scalar HWDGE ring (sync ring
            # stays exclusively output DMAs), ordered by first use
            chunk_order = [
                next(i for i, lv in enumerate(LEVELS) if lv[0] == l)
                for l in (3, 2, 1, 0)
            ]
            for li_ in chunk_order:
                off, ng, xoff, w = CHUNKS[li_]
                nc.scalar.dma_start(
                    out=sc[:, off : xoff + w], in_=scal[:, off : xoff + w]
                )

            for kind, li_, t in _schedule():
                lvl, h, w, stride, G8, PP, SL = LEVELS[li_]
                off, ng, xoff, _w = CHUNKS[li_]
                if kind == "p":  # packed level: one buffer, one DMA
                    buf = cpool.tile(
                        [PP, SL * 4 * w], _F32, tag=f"pk{lvl}", name=f"pk{lvl}"
                    )
                    for j in range(SL):
                        tv = buf[:, j * 4 * w : (j + 1) * 4 * w].rearrange(
                            "p (x c) -> p x c", c=4
                        )
                        fill(tv, sc[:PP, xoff : xoff + w], sc, off + 3 * j, PP)
                    nc.sync.dma_start(
                        out=outs[lvl][:, :].rearrange(
                            "(p s) x -> p (s x)", s=SL
                        ),
                        in_=buf[:, :],
                    )
                else:  # one TP-row-group tile
                    P = min(TP, G8 - t * TP)
                    tile = wpool.tile(
                        [128, 4 * w], _F32, tag="buf", name="buf"
                    )
                    tv = tile[:P, :].rearrange("p (x c) -> p x c", c=4)
                    fill(tv, sc[:P, xoff : xoff + w], sc, off + 3 * t, P)
                    nc.sync.dma_start(
                        out=outs[lvl][t * TP : t * TP + P, :],
                        in_=tile[:P, :],
                    )
    nc.finalize()
    return nc


def _tables(scales, ratios):
    """Host-precomputed per-core scalar tables [8, 128, SCALW] f32."""
    scales = np.asarray(scales, dtype=np.float32)
    ratios = np.asarray(ratios, dtype=np.float32)
    tabs = np.zeros((NCORES, 128, SCALW), np.float32)
    for li, (lvl, h, w, stride, G8, PP, SL) in enumerate(LEVELS):
        off, ng, xoff, _w = CHUNKS[li]
        xc = (np.arange(w) * stride + stride // 2).astype(np.float32)
        tabs[:, :, xoff : xoff + w] = xc[None, None, :]
        base = scales * np.float32(stride)               # [S] f32
        sr = np.sqrt(ratios)                             # [R] f32
        aw = (base[:, None] * sr[None, :]).reshape(-1)   # [A] f32
        ah = (base[:, None] / sr[None, :]).reshape(-1)   # [A] f32
        p = np.arange(128)
        for k in range(NCORES):
            for g_idx in range(ng):
                if SL > 1:
                    g_loc = p * SL + g_idx        # partition p, slot g_idx
                else:
                    g_loc = g_idx * TP + p        # tile g_idx, partition p
                g = k * G8 + np.minimum(g_loc, G8 - 1)
                a = g // h
                y = g % h
                c0 = off + 3 * g_idx
                tabs[k, :, c0] = (y * stride + stride // 2).astype(np.float32)
                tabs[k, :, c0 + 1] = aw[a]
                tabs[k, :, c0 + 2] = ah[a]
    return tabs


def _get_built():
    global _BUILT
    if _BUILT is None:
        _BUILT = _build()
    return _BUILT


def _run(scales, ratios, **spmd_kwargs):
    nc = _get_built()
    tabs = _tables(scales, ratios)
    in_maps = [{"scal": tabs[k]} for k in range(NCORES)]
    res = run_bass_kernel_spmd(nc, in_maps, core_ids=list(range(NCORES)),
                               **spmd_kwargs)
    outs = [None] * 4
    for (lvl, h, w, stride, G8, PP, SL) in LEVELS:
        full = np.concatenate(
            [res.results[k][f"out{lvl}"] for k in range(NCORES)], axis=0
        )
        outs[lvl] = full.reshape(A * h * w, 4)
    return tuple(outs), res


def kernel(scales, ratios, fs0_h, fs0_w, fs1_h, fs1_w, fs2_h, fs2_w,
           fs3_h, fs3_w):
    sizes = [(int(fs0_h), int(fs0_w)), (int(fs1_h), int(fs1_w)),
             (int(fs2_h), int(fs2_w)), (int(fs3_h), int(fs3_w))]
    assert sizes == SIZES, f"kernel compiled for {SIZES}, got {sizes}"
    outs, _ = _run(scales, ratios)
    return outs


# revision 21
# speedup vs baseline: 3.0688x; 3.0688x over previous
"""AnchorGenerator Bass kernel for 8 Trainium2 NeuronCores.

Generates multi-level anchor boxes: for each of 4 feature-map levels
(stride 4/8/16/32, sizes 1024^2/512^2/256^2/128^2), the output is
[9*h*w, 4] f32 rows (cx, cy, aw, ah), ordered (anchor_type, y, x).

Strategy (memory-bound, pure output generation ~200MB):
  - Shard each level's flattened rows contiguously across the 8 cores:
    core k owns rows [k*N/8, (k+1)*N/8). Host gather is a plain
    concatenate.
  - Per core+level the shard is [G8 row-groups, 4w floats]. Large levels
    are emitted as [128, 4w] tiles (2MB DMAs for level 0); the two small
    levels are packed into a single SBUF buffer each ([PP, S*4w]: each
    partition holds S consecutive row-groups) so each goes out as one
    DMA.
  - Tile content: col0 = xc (x centers, from the table input), cols
    1/2/3 (y center / anchor w / anchor h) are per-row-group constants
    written via per-partition-scalar ops (vector engine carries 3 of
    the 4 column writes, scalar engine 1).
  - Levels are emitted smallest-first so the small (slow, per-byte)
    transfers overlap the pipeline ramp and the kernel tail is big
    fast DMAs.
"""

import numpy as np

import concourse.bass as bass
import concourse.bacc as bacc
import concourse.mybir as mybir
from concourse.tile import TileContext
from concourse.bass_utils import run_bass_kernel_spmd

NCORES = 8
STRIDES = [4, 8, 16, 32]
SIZES = [(1024, 1024), (512, 512), (256, 256), (128, 128)]
S = 3  # scales
R = 3  # ratios
A = S * R

# Per-level derived constants, in EMISSION order (small -> large):
#   (lvl, h, w, stride, G8, PP, SLOTS)
#   G8 = row-groups (of w rows) per core; the shard is laid out as
#   [PP partitions, SLOTS row-groups each]; PP*SLOTS == G8 for packed
#   levels; SLOTS==1 means 128-partition tiling with T=ceil(G8/128).
_PLAN = [  # lvl, PP, SLOTS
    (3, 72, 2),    # G8=144  -> one [72, 4KB] buffer, one DMA
    (2, 96, 3),    # G8=288  -> one [96, 12KB] buffer, one DMA
    (1, 128, 1),   # G8=576  -> 5 [128, 8KB] tiles
    (0, 128, 1),   # G8=1152 -> 9 [128, 16KB] tiles
]
LEVELS = []
for _lvl, _PP, _SL in _PLAN:
    _h, _w = SIZES[_lvl]
    _G8 = A * _h // NCORES
    assert _h % 128 == 0 and (_SL == 1 or _PP * _SL == _G8)
    LEVELS.append((_lvl, _h, _w, STRIDES[_lvl], _G8, _PP, _SL))

# Tiled (SLOTS==1) levels use full-128-partition tiles: the SDMA
# descriptor balancer only spreads across all 16 engines for the
# 128-partition shape (124 was measured to collapse onto 4 engines).
TP = 128

# Table input layout: per level (emission order) a contiguous chunk:
#   [3 scalar cols per column-group ... | xc (w values)]
# where a column-group is one (tile) for SLOTS==1 levels (T of them) or
# one slot for packed levels (SLOTS of them).
CHUNKS = []  # (offset, ngroups, xc_off, width)
_o = 0
for (_lvl, _h, _w, _s, _G8, _PP, _SL) in LEVELS:
    _ng = _SL if _SL > 1 else (_G8 + TP - 1) // TP
    CHUNKS.append((_o, _ng, _o + 3 * _ng, _w))
    _o += 3 * _ng + _w
SCALW = _o

NBUFS = 9  # work-tile slots for the SLOTS==1 levels

_F32 = mybir.dt.float32

_BUILT = None  # cached build


def _build():
    """Build the Bass program (identical on all 8 cores; per-core data
    arrives via the small scalar-table input)."""
    nc = bacc.Bacc()

    scal = nc.dram_tensor("scal", [128, SCALW], _F32, kind="ExternalInput")
    outs = {}
    for (lvl, h, w, stride, G8, PP, SL) in LEVELS:
        outs[lvl] = nc.dram_tensor(
            f"out{lvl}", [G8, 4 * w], _F32, kind="ExternalOutput"
        )

    ident = mybir.ActivationFunctionType.Identity
    mult, add = mybir.AluOpType.mult, mybir.AluOpType.add

    def fill(tv, xcs, sc, c0, P):
        """Write one row-group-column: col0=xc, col1/2/3 = per-partition
        constants from table columns c0..c0+2."""
        nc.vector.tensor_copy(tv[:, :, 0], xcs)
        nc.vector.tensor_scalar(
            tv[:, :, 1], xcs, 0.0, sc[:P, c0 : c0 + 1], mult, add
        )
        nc.vector.tensor_scalar(
            tv[:, :, 2], xcs, 0.0, sc[:P, c0 + 1 : c0 + 2], mult, add
        )
        nc.scalar.activation(
            tv[:, :, 3], xcs, ident, bias=sc[:P, c0 + 2 : c0 + 3], scale=0.0
        )

    # Emission schedule: front-load big L0 tiles so the DMA queue
    # saturates with 2MB transfers as early as possible; interleave L1
    # and drop the packed L2/L3 buffers mid-stream.
    def _schedule():
        li = {lvl: i for i, (lvl, *_rest) in enumerate(LEVELS)}
        nt = {lvl: (G8 + TP - 1) // TP
              for (lvl, h, w, s, G8, PP, SL) in LEVELS}
        sched = [("p", li[3], 0), ("p", li[2], 0)]
        # partial (last) tiles first so their lower-parallelism transfers
        # land during the ramp, not the saturated middle/tail
        sched += [("t", li[1], t)
                  for t in [nt[1] - 1] + list(range(nt[1] - 1))]
        sched += [("t", li[0], t)
                  for t in [nt[0] - 1] + list(range(nt[0] - 1))]
        return sched

    with TileContext(nc) as tc:
        with (
            tc.tile_pool(name="consts", bufs=1) as cpool,
            tc.tile_pool(name="work", bufs=NBUFS) as wpool,
        ):
            sc = cpool.tile([128, SCALW], _F32, tag="sc", name="sc")
            # per-level table chunks on the scalar HWDGE ring (sync ring
            # stays exclusively output DMAs), ordered by first use
            chunk_order = [
                next(i for i, lv in enumerate(LEVELS) if lv[0] == l)
                for l in (3, 2, 1, 0)
            ]
            for li_ in chunk_order:
                off, ng, xoff, w = CHUNKS[li_]
                nc.scalar.dma_start(
                    out=sc[:, off : xoff + w], in_=scal[:, off : xoff + w]
                )

            for kind, li_, t in _schedule():
                lvl, h, w, stride, G8, PP, SL = LEVELS[li_]
                off, ng, xoff, _w = CHUNKS[li_]
                if kind == "p":  # packed level: one buffer, one DMA
                    buf = cpool.tile(
                        [PP, SL * 4 * w], _F32, tag=f"pk{lvl}", name=f"pk{lvl}"
                    )
                    for j in range(SL):
                        tv = buf[:, j * 4 * w : (j + 1) * 4 * w].rearrange(
                            "p (x c) -> p x c", c=4
                        )
                        fill(tv, sc[:PP, xoff : xoff + w], sc, off + 3 * j, PP)
                    nc.sync.dma_start(
                        out=outs[lvl][:, :].rearrange(
                            "(p s) x -> p (s x)", s=SL
                        ),
                        in_=buf[:, :],
                    )
                else:  # one TP-row-group tile
                    P = min(TP, G8 - t * TP)
                    tile = wpool.tile(
                        [128, 4 * w], _F32, tag="buf", name="buf"
                    )
                    tv = tile[:P, :].rearrange("p (x c) -> p x c", c=4)
                    fill(tv, sc[:P, xoff : xoff + w], sc, off + 3 * t, P)
                    nc.sync.dma_start(
                        out=outs[lvl][t * TP : t * TP + P, :],
                        in_=tile[:P, :],
                    )
    nc.finalize()
    return nc


def _tables(scales, ratios):
    """Host-precomputed per-core scalar tables [8, 128, SCALW] f32."""
    scales = np.asarray(scales, dtype=np.float32)
    ratios = np.asarray(ratios, dtype=np.float32)
    tabs = np.zeros((NCORES, 128, SCALW), np.float32)
    for li, (lvl, h, w, stride, G8, PP, SL) in enumerate(LEVELS):
        off, ng, xoff, _w = CHUNKS[li]
        xc = (np.arange(w) * stride + stride // 2).astype(np.float32)
        tabs[:, :, xoff : xoff + w] = xc[None, None, :]
        base = scales * np.float32(stride)               # [S] f32
        sr = np.sqrt(ratios)                             # [R] f32
        aw = (base[:, None] * sr[None, :]).reshape(-1)   # [A] f32
        ah = (base[:, None] / sr[None, :]).reshape(-1)   # [A] f32
        p = np.arange(128)
        for k in range(NCORES):
            for g_idx in range(ng):
                if SL > 1:
                    g_loc = p * SL + g_idx        # partition p, slot g_idx
                else:
                    g_loc = g_idx * TP + p        # tile g_idx, partition p
                g = k * G8 + np.minimum(g_loc, G8 - 1)
                a = g // h
                y = g % h
                c0 = off + 3 * g_idx
                tabs[k, :, c0] = (y * stride + stride // 2).astype(np.float32)
                tabs[k, :, c0 + 1] = aw[a]
                tabs[k, :, c0 + 2] = ah[a]
    return tabs


def _get_built():
    global _BUILT
    if _BUILT is None:
        _BUILT = _build()
    return _BUILT


def _run(scales, ratios, **spmd_kwargs):
    nc = _get_built()
    tabs = _tables(scales, ratios)
    in_maps = [{"scal": tabs[k]} for k in range(NCORES)]
    res = run_bass_kernel_spmd(nc, in_maps, core_ids=list(range(NCORES)),
                               **spmd_kwargs)
    outs = [None] * 4
    for (lvl, h, w, stride, G8, PP, SL) in LEVELS:
        full = np.concatenate(
            [res.results[k][f"out{lvl}"] for k in range(NCORES)], axis=0
        )
        outs[lvl] = full.reshape(A * h * w, 4)
    return tuple(outs), res


def kernel(scales, ratios, fs0_h, fs0_w, fs1_h, fs1_w, fs2_h, fs2_w,
           fs3_h, fs3_w):
    sizes = [(int(fs0_h), int(fs0_w)), (int(fs1_h), int(fs1_w)),
             (int(fs2_h), int(fs2_w)), (int(fs3_h), int(fs3_w))]
    assert sizes == SIZES, f"kernel compiled for {SIZES}, got {sizes}"
    outs, _ = _run(scales, ratios)
    return outs
